# revision 9
# baseline (speedup 1.0000x reference)
"""Trainium2 Bass kernel for DLRA conv layer (3x3 low-rank conv + bias + relu).

Computes: relu(extract_patches_3x3(x) @ U @ W2 + bias) for the step-selected
factor set (W2 = S @ V folded on host for step 2). Sharded over H across 8
NeuronCores (28 rows each, 1-px halo resolved on host).

Device dataflow per core, per image (v2 — engine-rebalanced):
  stage 1 (576->100): per quad (2 row-pairs A/B = 4 output rows, 896 px):
    - 3 full-array K=128 matmuls per row-pair contract shift pairs
      (di,0)+(di,1) via bufA = [x ; x<<1col] (2 HBM loads, no SBUF shift).
    - the 3 leftover dj=2 singles are K=64 and run as CONCURRENT row-tiled
      matmul pairs (tile_position (0,0)/(64,0)) -> 9 PE slots per 896 px.
    - ACT drains psum -> per-quad z1 tiles (fp16, [100 x 896]); ACT does
      nothing else (~1us/quad), so drains complete inside the next quad's
      pair phase and stage-2 runs at a 1-quad lag without backpressure.
  stage 2 (100->256): one psum tile [128 x 1024] per (quad, filter-half)
    holding two N=448 matmuls (cols 0:448 / 512:960 -- matmul writes may
    not cross a psum bank). A ready-queue of drained quads is consumed
    down to a 1-quad lag after each quad's stage-1.
  epilogue: the bias-add reads PSUM directly on DVE (the only non-ACT
    engine with PSUM read access; gpsimd cannot touch PSUM and its vector
    ops are ~30x slower than DVE anyway), strided [128,(2,448)] against the
    contiguous bias slice, then an in-place per-quad fp16 relu at DVE 4x
    mode. og strips store per fh in 2 chunks (gpsimd queue for fh0, scalar
    queue for fh1); the last image stores per quad so the post-matmul tail
    is ~2us.

  Scheduling: x0 loads in 2 row-chunks so quad 0 can start ~4us earlier than
  a monolithic load; only 3 dummy warm-up matmuls are needed to keep the PE
  HAM clock-gate counter alive until real work arrives. The bias rides the
  scalar DMA queue in 4 chunks behind w1/w2 (sync queue stays exclusive to
  x strips). Host transposes [fh,f,px] -> (H,W,256) and casts fp32.
"""

import numpy as np
from contextlib import ExitStack

import concourse.bacc as bacc
import concourse.tile as tile
import concourse.mybir as mybir
from concourse.bass_utils import run_bass_kernel_spmd

B, H, W, C = 8, 224, 224, 64
KH = KW = 3
RANK = 100
FILTERS = 256
IN_DIM = KH * KW * C  # 576

NCORES = 8
HS = H // NCORES          # 28 output rows per core
HSH = HS + 2              # input rows incl halo
WP = W + 2                # padded width
XL = HSH * WP             # flat image-strip length per channel (6780)
NPIX = HS * W             # 6272 pixels per image strip
RPP = 2 * W               # 448 px per row-pair (stage-1 matmul N)
NQ = HS // 4              # 7 quads (2 row-pairs each) per image
NRP = HS // 2             # 14 row-pairs per image
MP = 128                  # padded stationary columns (rank 100 -> 128)
X0_ROWS = 10              # first-chunk padded rows of image 0

N_WARMUP_MM = 10

F32 = mybir.dt.float32
MM_DT = mybir.dt.float16
MM_NP = np.float16

_CACHE = {}


def _build_nc():
    nc = bacc.Bacc("TRN2", target_bir_lowering=False, debug=False,
                   num_devices=NCORES)
    xt = nc.dram_tensor("xt", [B, C, XL], MM_DT, kind="ExternalInput").ap()
    w1 = nc.dram_tensor("w1", [2 * C, 6 * MP], MM_DT,
                        kind="ExternalInput").ap()
    w2 = nc.dram_tensor("w2", [RANK, FILTERS], MM_DT,
                        kind="ExternalInput").ap()
    bias = nc.dram_tensor("bias", [MP, 2 * NPIX], MM_DT,
                          kind="ExternalInput").ap()
    out = nc.dram_tensor("out", [B, 2, MP, NPIX], MM_DT,
                         kind="ExternalOutput").ap()
    fcopy = mybir.ActivationFunctionType.Copy
    add = mybir.AluOpType.add
    QPX = 2 * RPP              # 896 px per quad
    RELU_CUT = 3 * 1024        # og relu/store chunk boundary

    with tile.TileContext(nc) as tc, ExitStack() as ctx:
        const = ctx.enter_context(tc.tile_pool(name="const", bufs=1))
        xpool = ctx.enter_context(tc.tile_pool(name="xpool", bufs=2))
        z1pool = ctx.enter_context(tc.tile_pool(name="z1pool", bufs=2))
        ps1pool = ctx.enter_context(
            tc.tile_pool(name="ps1", bufs=2, space="PSUM"))
        ps2pool = ctx.enter_context(
            tc.tile_pool(name="ps2", bufs=2, space="PSUM"))
        ogpool = ctx.enter_context(tc.tile_pool(name="ogpool", bufs=4))

        # consts on the scalar queue: w1 first (stage-1 needs it by the
        # first pair matmul), then w2 and the bias in 4 chunks (fh0a,
        # fh1a, fh0b, fh1b) so both filter-halves' early columns land
        # before image 0's first stage-2 bias-add.
        w1_t = const.tile([2 * C, 6 * MP], MM_DT, name="w1_t")
        nc.scalar.dma_start(w1_t[:], w1[:])
        w2_t = const.tile([RANK, FILTERS], MM_DT, name="w2_t")
        nc.scalar.dma_start(w2_t[:], w2[:])
        bias_t = const.tile([MP, 2 * NPIX], MM_DT, name="bias_t")
        HPX = NPIX // 2
        for fh in range(2):
            nc.scalar.dma_start(bias_t[:, fh * NPIX:fh * NPIX + HPX],
                                bias[:, fh * NPIX:fh * NPIX + HPX])
        for fh in range(2):
            nc.scalar.dma_start(bias_t[:, fh * NPIX + HPX:(fh + 1) * NPIX],
                                bias[:, fh * NPIX + HPX:(fh + 1) * NPIX])

        # HAM warm-up: bridge the PE from ~t=0 until x0's first rows and
        # w1 land (~10us: each DMA queue has ~2.6us bring-up and then
        # moves ~140GB/s), so the clock-gate ramp counter never resets.
        dummy = const.tile([MP, 512], MM_DT, name="dummy")
        nc.gpsimd.memset(dummy[:], 0.0)
        for _ in range(N_WARMUP_MM):
            psd = ps1pool.tile([MP, 1024], F32, name="psd", tag="psAB")
            nc.tensor.matmul(psd[:, 0:512], lhsT=dummy[:, 0:MP],
                             rhs=dummy[:],
                             start=True, stop=True, skip_group_check=True)

        def load_image(img, chunked=False):
            """bufA = [x ; x shifted 1 col]. The hi half rides the sync
            queue and the lo (shifted) half the gpsimd queue -- one HW DMA
            queue sustains only ~140GB/s, two in parallel halve the image
            load latency. Tail element of the lo half stays garbage: it is
            never read. Image 0 additionally splits each half into 2
            row-chunks so quad 0's rows arrive ~5us sooner."""
            bufa = xpool.tile([2 * C, XL], MM_DT, name="bufa", tag="bufa")
            if chunked:
                cut = X0_ROWS * WP
                nc.sync.dma_start(bufa[0:C, 0:cut], xt[img, :, 0:cut])
                nc.gpsimd.dma_start(bufa[C:2 * C, 0:cut],
                                    xt[img, :, 1:cut + 1])
                nc.sync.dma_start(bufa[0:C, cut:XL], xt[img, :, cut:XL])
                nc.gpsimd.dma_start(bufa[C:2 * C, cut:XL - 1],
                                    xt[img, :, cut + 1:XL])
            else:
                nc.sync.dma_start(bufa[0:C, :], xt[img])
                nc.gpsimd.dma_start(bufa[C:2 * C, 0:XL - 1],
                                    xt[img, :, 1:XL])
            return bufa

        def stage1_quad(bufa, q, z1t):
            """Conv 576->100 for quad q (row-pairs 2q, 2q+1; 896 px)."""
            av = bufa[:, 0:XL].rearrange("c (r w) -> c r w", w=WP)
            rA = 4 * q          # padded-row base of row-pair A
            rB = 4 * q + 2
            psAB = ps1pool.tile([MP, 1024], F32, name="psAB", tag="psAB")
            psA = psAB[:, 0:RPP]
            psB = psAB[:, 512:512 + RPP]
            for p in range(3):  # shift pairs (p,0)+(p,1), K=128
                lw = w1_t[:, p * MP:(p + 1) * MP]
                nc.tensor.matmul(psA, lhsT=lw,
                                 rhs=av[:, rA + p:rA + p + 2, 0:W],
                                 start=(p == 0), stop=False,
                                 skip_group_check=True)
                nc.tensor.matmul(psB, lhsT=lw,
                                 rhs=av[:, rB + p:rB + p + 2, 0:W],
                                 start=(p == 0), stop=False,
                                 skip_group_check=True)
            for s in range(3):  # singles (s,2), K=64, concurrent row-tiles
                sl = (3 + s) * MP
                last = (s == 2)
                nc.tensor.matmul(psA, lhsT=w1_t[0:C, sl:sl + MP],
                                 rhs=av[0:C, rA + s:rA + s + 2, 2:2 + W],
                                 start=False, stop=last,
                                 skip_group_check=True, tile_position=(0, 0))
                nc.tensor.matmul(psB, lhsT=w1_t[C:2 * C, sl:sl + MP],
                                 rhs=av[C:2 * C, rB + s:rB + s + 2, 1:1 + W],
                                 start=False, stop=last,
                                 skip_group_check=True, tile_position=(C, 0))
            # per-quad z1 drain into the per-image z1 strip (the only ACT
            # work in the kernel)
            zdst = z1t[:, q * QPX:(q + 1) * QPX]
            zsrc = psAB[0:RANK, :].rearrange("r (a b) -> r a b", b=512)
            nc.scalar.activation(
                zdst.rearrange("r (a b) -> r a b", b=RPP),
                zsrc[:, :, 0:RPP], fcopy)

        ogs = {}  # (img, fh) -> og tile

        def get_og(img, fh):
            if (img, fh) not in ogs:
                ogs[(img, fh)] = ogpool.tile([MP, NPIX], MM_DT, name="og",
                                             tag="og")
            return ogs[(img, fh)]

        # stage-2 tiles per (img, fh): 6 x 1024 cols + one 128-col rag.
        NT2 = 7

        def t2cols(t):
            base = 1024 * t
            return base, (1024 if t < NT2 - 1 else NPIX - 6144)

        def s2_tile(img, fh, t, z1t, fine=False):
            """100->256 for one (fh, col tile t) + epilogue.

            Bias-adds read PSUM on DVE (the only non-ACT engine with PSUM
            access; gpsimd's vector ops are ~30x slower than DVE). Relus
            run batched on DVE at 4x fp16 mode when each og chunk
            completes; `fine` (last image) relus+stores per tile so the
            post-matmul tail stays ~2us."""
            base, width = t2cols(t)
            og = get_og(img, fh)
            lw2 = w2_t[:, fh * MP:(fh + 1) * MP]
            ps2 = ps2pool.tile([MP, 1024], F32, name="ps2", tag="ps2")
            for o in range(0, width, 512):
                nn = min(512, width - o)
                nc.tensor.matmul(ps2[:, o:o + nn], lhsT=lw2,
                                 rhs=z1t[:, base + o:base + o + nn],
                                 start=True, stop=True, skip_group_check=True)
            oc = og[:, base:base + width]
            bc = bias_t[:, fh * NPIX + base:fh * NPIX + base + width]
            nc.vector.tensor_tensor(oc, ps2[:, 0:width], bc, add)
            if fine:
                nc.vector.tensor_scalar_max(oc, oc, 0.0)
                deng = nc.gpsimd if fh == 0 else nc.scalar
                deng.dma_start(out[img, fh, :, base:base + width], oc)
                if t == NT2 - 1:
                    del ogs[(img, fh)]
                return
            # batched relu + store at the two chunk boundaries. Store
            # queues are spread so no queue carries more than ~1.7MB per
            # image: fh0a->gpsimd, fh1a->sync, fh0b/fh1b->scalar.
            if base + width == RELU_CUT:
                nc.vector.tensor_scalar_max(og[:, 0:RELU_CUT],
                                            og[:, 0:RELU_CUT], 0.0)
                deng = nc.gpsimd if fh == 0 else nc.sync
                deng.dma_start(out[img, fh, :, 0:RELU_CUT],
                               og[:, 0:RELU_CUT])
            elif t == NT2 - 1:
                nc.vector.tensor_scalar_max(og[:, RELU_CUT:NPIX],
                                            og[:, RELU_CUT:NPIX], 0.0)
                nc.scalar.dma_start(out[img, fh, :, RELU_CUT:NPIX],
                                    og[:, RELU_CUT:NPIX])
                del ogs[(img, fh)]

        # ---- schedule ----
        # After quad q of image i drains, stage-2 tiles needing cols
        # <= 896*q (a 1-quad lag) are emitted; the 2 tiles per fh left
        # over at image end ride the next image's quad-0 slot (the last
        # image drains them in `fine` mode right after its last quad).
        pending = []  # (img, t, z1t) not yet emitted, in col order
        done_upto = {}  # img -> drained cols

        def consume(img_avail, avail, fine=False):
            while pending:
                img, t, z1t = pending[0]
                base, width = t2cols(t)
                need = base + width
                limit = avail if img == img_avail else NPIX
                if need > limit:
                    break
                for fh in range(2):
                    # fine (per-tile stores) only for the LAST image's own
                    # tiles; an earlier image's trailing tiles consumed in
                    # the same slot keep their batched store triggers.
                    s2_tile(img, fh, t, z1t, fine=fine and img == B - 1)
                pending.pop(0)

        bufa_cur = load_image(0, chunked=True)
        for img in range(B):
            bufa_next = load_image(img + 1) if img + 1 < B else None
            z1t = z1pool.tile([RANK, NPIX], MM_DT, name="z1", tag="z1")
            for t in range(NT2):
                pending.append((img, t, z1t))
            fine = (img == B - 1)
            for q in range(NQ):
                stage1_quad(bufa_cur, q, z1t)
                consume(img, QPX * q, fine=fine)
            bufa_cur = bufa_next
        consume(B - 1, NPIX, fine=True)  # last image's trailing tiles

    nc.compile()
    return nc


def _get_nc():
    if "nc" not in _CACHE:
        _CACHE["nc"] = _build_nc()
    return _CACHE["nc"]


def _prep_inputs(x, k, l_t, s, aux_U, aux_Unp1, aux_Vt, aux_Vtnp1, b, aux_b,
                 step):
    step = int(np.asarray(step))
    x = np.ascontiguousarray(np.asarray(x, dtype=np.float32))
    if step == 0:
        U, W2, bias = np.asarray(k), np.asarray(aux_Vt), np.asarray(aux_b)
    elif step == 1:
        U, W2, bias = np.asarray(aux_U), np.asarray(l_t), np.asarray(aux_b)
    else:
        U = np.asarray(aux_Unp1)
        W2 = (np.asarray(s, np.float64) @ np.asarray(aux_Vtnp1, np.float64))
        bias = np.asarray(b)
    U = U.astype(np.float32)
    W2 = np.ascontiguousarray(W2.astype(MM_NP))
    bias = np.asarray(bias, np.float32)

    # channel-major, zero-padded H and W, fp16
    xpad = np.zeros((B, H + 2, W + 2, C), np.float32)
    xpad[:, 1:-1, 1:-1, :] = x
    xpad_t = np.ascontiguousarray(xpad.transpose(0, 3, 1, 2)).astype(MM_NP)

    # stage-1 stationary slots [128, 6*128]:
    #   p=0..2: top=blocks[p,0], bottom=blocks[p,1] (pairs, K=128)
    #   p=3..5: blocks[p-3,2] duplicated into both halves (concurrent K=64
    #           row-tiles for row-pairs A and B)
    blocks = U.reshape(KH, KW, C, RANK)
    w1p = np.zeros((6, 2 * C, MP), np.float32)
    for p in range(3):
        w1p[p, 0:C, 0:RANK] = blocks[p, 0]
        w1p[p, C:2 * C, 0:RANK] = blocks[p, 1]
    for s_ in range(3):
        w1p[3 + s_, 0:C, 0:RANK] = blocks[s_, 2]
        w1p[3 + s_, C:2 * C, 0:RANK] = blocks[s_, 2]
    w1 = np.ascontiguousarray(
        w1p.transpose(1, 0, 2).reshape(2 * C, 6 * MP)).astype(MM_NP)

    in_maps = []
    for i in range(NCORES):
        xt_i = np.ascontiguousarray(
            xpad_t[:, :, HS * i:HS * i + HSH, :]).reshape(B, C, XL)
        # bias strip -> [f, fh*NPIX + px] (transposed, filter-major)
        bs = bias[HS * i:HS * (i + 1)].reshape(NPIX, FILTERS)
        bt = np.ascontiguousarray(bs.T).astype(MM_NP)      # (256, NPIX)
        b_i = np.ascontiguousarray(
            np.concatenate([bt[0:MP], bt[MP:FILTERS]], axis=1))
        in_maps.append({"xt": xt_i, "w1": w1, "w2": W2, "bias": b_i})
    return in_maps


def _assemble(results):
    strips = [
        results[i]["out"].transpose(0, 3, 1, 2).reshape(B, HS, W, FILTERS)
        for i in range(NCORES)
    ]
    return np.concatenate(strips, axis=1).astype(np.float32)


def run(trace=False, **inputs):
    in_maps = _prep_inputs(**inputs)
    nc = _get_nc()
    res = run_bass_kernel_spmd(nc, in_maps, list(range(NCORES)), trace=trace)
    return _assemble(res.results), res


def kernel(**inputs):
    out, _ = run(trace=False, **inputs)
    return out


# revision 10
# speedup vs baseline: 1.0123x; 1.0123x over previous
"""Trainium2 Bass kernel for DLRA conv layer (3x3 low-rank conv + bias + relu).

Computes: relu(extract_patches_3x3(x) @ U @ W2 + bias) for the step-selected
factor set (W2 = S @ V folded on host for step 2). Sharded over H across 8
NeuronCores (28 rows each, 1-px halo resolved on host).

Device dataflow per core, per image (v2 — engine-rebalanced):
  stage 1 (576->100): per quad (2 row-pairs A/B = 4 output rows, 896 px):
    - 3 full-array K=128 matmuls per row-pair contract shift pairs
      (di,0)+(di,1) via bufA = [x ; x<<1col] (2 HBM loads, no SBUF shift).
    - the 3 leftover dj=2 singles are K=64 and run as CONCURRENT row-tiled
      matmul pairs (tile_position (0,0)/(64,0)) -> 9 PE slots per 896 px.
    - ACT drains psum -> per-quad z1 tiles (fp16, [100 x 896]); ACT does
      nothing else (~1us/quad), so drains complete inside the next quad's
      pair phase and stage-2 runs at a 1-quad lag without backpressure.
  stage 2 (100->256): one psum tile [128 x 1024] per (quad, filter-half)
    holding two N=448 matmuls (cols 0:448 / 512:960 -- matmul writes may
    not cross a psum bank). A ready-queue of drained quads is consumed
    down to a 1-quad lag after each quad's stage-1.
  epilogue: the bias-add reads PSUM directly on DVE (the only non-ACT
    engine with PSUM read access; gpsimd cannot touch PSUM and its vector
    ops are ~30x slower than DVE anyway), strided [128,(2,448)] against the
    contiguous bias slice, then an in-place per-quad fp16 relu at DVE 4x
    mode. og strips store per fh in 2 chunks (gpsimd queue for fh0, scalar
    queue for fh1); the last image stores per quad so the post-matmul tail
    is ~2us.

  Scheduling: x0 loads in 2 row-chunks so quad 0 can start ~4us earlier than
  a monolithic load; only 3 dummy warm-up matmuls are needed to keep the PE
  HAM clock-gate counter alive until real work arrives. The bias rides the
  scalar DMA queue in 4 chunks behind w1/w2 (sync queue stays exclusive to
  x strips). Host transposes [fh,f,px] -> (H,W,256) and casts fp32.
"""

import numpy as np
from contextlib import ExitStack

import concourse.bacc as bacc
import concourse.tile as tile
import concourse.mybir as mybir
from concourse.bass_utils import run_bass_kernel_spmd

B, H, W, C = 8, 224, 224, 64
KH = KW = 3
RANK = 100
FILTERS = 256
IN_DIM = KH * KW * C  # 576

NCORES = 8
HS = H // NCORES          # 28 output rows per core
HSH = HS + 2              # input rows incl halo
WP = W + 2                # padded width
XL = HSH * WP             # flat image-strip length per channel (6780)
NPIX = HS * W             # 6272 pixels per image strip
RPP = 2 * W               # 448 px per row-pair (stage-1 matmul N)
NQ = HS // 4              # 7 quads (2 row-pairs each) per image
NRP = HS // 2             # 14 row-pairs per image
MP = 128                  # padded stationary columns (rank 100 -> 128)
X0_ROWS = 6               # first-chunk padded rows of image 0

N_WARMUP_MM = 8

F32 = mybir.dt.float32
MM_DT = mybir.dt.float16
MM_NP = np.float16

_CACHE = {}


def _build_nc():
    nc = bacc.Bacc("TRN2", target_bir_lowering=False, debug=False,
                   num_devices=NCORES)
    xt = nc.dram_tensor("xt", [B, C, XL], MM_DT, kind="ExternalInput").ap()
    w1 = nc.dram_tensor("w1", [2 * C, 6 * MP], MM_DT,
                        kind="ExternalInput").ap()
    w2 = nc.dram_tensor("w2", [RANK, FILTERS], MM_DT,
                        kind="ExternalInput").ap()
    bias = nc.dram_tensor("bias", [MP, 2 * NPIX], MM_DT,
                          kind="ExternalInput").ap()
    out = nc.dram_tensor("out", [B, 2, MP, NPIX], MM_DT,
                         kind="ExternalOutput").ap()
    fcopy = mybir.ActivationFunctionType.Copy
    add = mybir.AluOpType.add
    QPX = 2 * RPP              # 896 px per quad
    RELU_CUT = 3 * 1024        # og relu/store chunk boundary

    with tile.TileContext(nc) as tc, ExitStack() as ctx:
        const = ctx.enter_context(tc.tile_pool(name="const", bufs=1))
        xpool = ctx.enter_context(tc.tile_pool(name="xpool", bufs=2))
        z1pool = ctx.enter_context(tc.tile_pool(name="z1pool", bufs=2))
        ps1pool = ctx.enter_context(
            tc.tile_pool(name="ps1", bufs=2, space="PSUM"))
        ps2pool = ctx.enter_context(
            tc.tile_pool(name="ps2", bufs=2, space="PSUM"))
        ogpool = ctx.enter_context(tc.tile_pool(name="ogpool", bufs=4))
        t2pool = ctx.enter_context(tc.tile_pool(name="t2pool", bufs=2))

        # consts on the scalar queue: w1 first (stage-1 needs it by the
        # first pair matmul), then w2 and the bias in 4 chunks (fh0a,
        # fh1a, fh0b, fh1b) so both filter-halves' early columns land
        # before image 0's first stage-2 bias-add.
        w1_t = const.tile([2 * C, 6 * MP], MM_DT, name="w1_t")
        nc.scalar.dma_start(w1_t[:], w1[:])
        w2_t = const.tile([RANK, FILTERS], MM_DT, name="w2_t")
        nc.scalar.dma_start(w2_t[:], w2[:])
        bias_t = const.tile([MP, 2 * NPIX], MM_DT, name="bias_t")
        HPX = NPIX // 2
        for fh in range(2):
            nc.scalar.dma_start(bias_t[:, fh * NPIX:fh * NPIX + HPX],
                                bias[:, fh * NPIX:fh * NPIX + HPX])
        for fh in range(2):
            nc.scalar.dma_start(bias_t[:, fh * NPIX + HPX:(fh + 1) * NPIX],
                                bias[:, fh * NPIX + HPX:(fh + 1) * NPIX])

        # HAM warm-up: bridge the PE from ~t=0 until x0's first rows and
        # w1 land (~10us: each DMA queue has ~2.6us bring-up and then
        # moves ~140GB/s), so the clock-gate ramp counter never resets.
        dummy = const.tile([MP, 512], MM_DT, name="dummy")
        nc.gpsimd.memset(dummy[:], 0.0)
        for _ in range(N_WARMUP_MM):
            psd = ps1pool.tile([MP, 1024], F32, name="psd", tag="psAB")
            nc.tensor.matmul(psd[:, 0:512], lhsT=dummy[:, 0:MP],
                             rhs=dummy[:],
                             start=True, stop=True, skip_group_check=True)

        def load_image(img, chunked=False):
            """bufA = [x ; x shifted 1 col]. The hi half rides the sync
            queue and the lo (shifted) half the gpsimd queue -- one HW DMA
            queue sustains only ~140GB/s, two in parallel halve the image
            load latency. Tail element of the lo half stays garbage: it is
            never read. Image 0 additionally splits each half into 2
            row-chunks so quad 0's rows arrive ~5us sooner."""
            bufa = xpool.tile([2 * C, XL], MM_DT, name="bufa", tag="bufa")
            if chunked:
                # quad 0's rows (both halves) ride sync alone -- one queue
                # bring-up (~2.6us) gates the first real matmul; the rest
                # splits across sync (hi) + gpsimd (lo).
                cut = X0_ROWS * WP
                nc.sync.dma_start(bufa[0:C, 0:cut], xt[img, :, 0:cut])
                nc.sync.dma_start(bufa[C:2 * C, 0:cut],
                                  xt[img, :, 1:cut + 1])
                nc.sync.dma_start(bufa[0:C, cut:XL], xt[img, :, cut:XL])
                nc.gpsimd.dma_start(bufa[C:2 * C, cut:XL - 1],
                                    xt[img, :, cut + 1:XL])
            else:
                nc.sync.dma_start(bufa[0:C, :], xt[img])
                nc.gpsimd.dma_start(bufa[C:2 * C, 0:XL - 1],
                                    xt[img, :, 1:XL])
            return bufa

        def stage1_quad(bufa, q, z1t):
            """Conv 576->100 for quad q (row-pairs 2q, 2q+1; 896 px)."""
            av = bufa[:, 0:XL].rearrange("c (r w) -> c r w", w=WP)
            rA = 4 * q          # padded-row base of row-pair A
            rB = 4 * q + 2
            psAB = ps1pool.tile([MP, 1024], F32, name="psAB", tag="psAB")
            psA = psAB[:, 0:RPP]
            psB = psAB[:, 512:512 + RPP]
            for p in range(3):  # shift pairs (p,0)+(p,1), K=128
                lw = w1_t[:, p * MP:(p + 1) * MP]
                nc.tensor.matmul(psA, lhsT=lw,
                                 rhs=av[:, rA + p:rA + p + 2, 0:W],
                                 start=(p == 0), stop=False,
                                 skip_group_check=True)
                nc.tensor.matmul(psB, lhsT=lw,
                                 rhs=av[:, rB + p:rB + p + 2, 0:W],
                                 start=(p == 0), stop=False,
                                 skip_group_check=True)
            for s in range(3):  # singles (s,2), K=64, concurrent row-tiles
                sl = (3 + s) * MP
                last = (s == 2)
                nc.tensor.matmul(psA, lhsT=w1_t[0:C, sl:sl + MP],
                                 rhs=av[0:C, rA + s:rA + s + 2, 2:2 + W],
                                 start=False, stop=last,
                                 skip_group_check=True, tile_position=(0, 0))
                nc.tensor.matmul(psB, lhsT=w1_t[C:2 * C, sl:sl + MP],
                                 rhs=av[C:2 * C, rB + s:rB + s + 2, 1:1 + W],
                                 start=False, stop=last,
                                 skip_group_check=True, tile_position=(C, 0))
            # per-quad z1 drain into the per-image z1 strip (the only ACT
            # work in the kernel)
            zdst = z1t[:, q * QPX:(q + 1) * QPX]
            zsrc = psAB[0:RANK, :].rearrange("r (a b) -> r a b", b=512)
            nc.scalar.activation(
                zdst.rearrange("r (a b) -> r a b", b=RPP),
                zsrc[:, :, 0:RPP], fcopy)

        ogs = {}  # (img, fh) -> og tile

        def get_og(img, fh):
            if (img, fh) not in ogs:
                ogs[(img, fh)] = ogpool.tile([MP, NPIX], MM_DT, name="og",
                                             tag="og")
            return ogs[(img, fh)]

        # stage-2 tiles per (img, fh): 6 x 1024 cols + one 128-col rag.
        NT2 = 7

        def t2cols(t):
            base = 1024 * t
            return base, (1024 if t < NT2 - 1 else NPIX - 6144)

        def s2_tile(img, fh, t, z1t, fine=False):
            """100->256 for one (fh, col tile t) + epilogue.

            PSUM can only be read by ACT and DVE, and one engine alone
            cannot keep up with the PE (psum reads run at 1x), so tiles
            alternate two drain paths: even tiles DVE-add straight from
            psum (fp32+fp16->fp16, ~1.1us), odd tiles go ACT fp32->fp16
            copy (~1.0us) then a cheap DVE 2x-mode sbuf add. Relus batch
            on DVE at 4x fp16 mode per og chunk; `fine` (last image)
            relus+stores per tile so the post-matmul tail stays short."""
            base, width = t2cols(t)
            og = get_og(img, fh)
            lw2 = w2_t[:, fh * MP:(fh + 1) * MP]
            ps2 = ps2pool.tile([MP, 1024], F32, name="ps2", tag="ps2")
            for o in range(0, width, 512):
                nn = min(512, width - o)
                nc.tensor.matmul(ps2[:, o:o + nn], lhsT=lw2,
                                 rhs=z1t[:, base + o:base + o + nn],
                                 start=True, stop=True, skip_group_check=True)
            oc = og[:, base:base + width]
            bc = bias_t[:, fh * NPIX + base:fh * NPIX + base + width]
            if (t + fh) % 2 == 1:
                t2 = t2pool.tile([MP, 1024], MM_DT, name="t2", tag="t2")
                nc.scalar.activation(t2[:, 0:width], ps2[:, 0:width], fcopy)
                nc.vector.tensor_tensor(oc, t2[:, 0:width], bc, add)
            else:
                nc.vector.tensor_tensor(oc, ps2[:, 0:width], bc, add)
            if fine:
                nc.vector.tensor_scalar_max(oc, oc, 0.0)
                deng = nc.gpsimd if fh == 0 else nc.scalar
                deng.dma_start(out[img, fh, :, base:base + width], oc)
                if t == NT2 - 1:
                    del ogs[(img, fh)]
                return
            # batched relu + store at the two chunk boundaries. Store
            # queues are spread so no queue carries more than ~1.7MB per
            # image: fh0a->gpsimd, fh1a->sync, fh0b/fh1b->scalar.
            if base + width == RELU_CUT:
                nc.vector.tensor_scalar_max(og[:, 0:RELU_CUT],
                                            og[:, 0:RELU_CUT], 0.0)
                deng = nc.gpsimd if fh == 0 else nc.sync
                deng.dma_start(out[img, fh, :, 0:RELU_CUT],
                               og[:, 0:RELU_CUT])
            elif t == NT2 - 1:
                nc.vector.tensor_scalar_max(og[:, RELU_CUT:NPIX],
                                            og[:, RELU_CUT:NPIX], 0.0)
                nc.scalar.dma_start(out[img, fh, :, RELU_CUT:NPIX],
                                    og[:, RELU_CUT:NPIX])
                del ogs[(img, fh)]

        # ---- schedule ----
        # After quad q of image i drains, stage-2 tiles needing cols
        # <= 896*q (a 1-quad lag) are emitted; the 2 tiles per fh left
        # over at image end ride the next image's quad-0 slot (the last
        # image drains them in `fine` mode right after its last quad).
        pending = []  # (img, t, z1t) not yet emitted, in col order
        done_upto = {}  # img -> drained cols

        def consume(img_avail, avail, fine=False):
            while pending:
                img, t, z1t = pending[0]
                base, width = t2cols(t)
                need = base + width
                limit = avail if img == img_avail else NPIX
                if need > limit:
                    break
                for fh in range(2):
                    # fine (per-tile stores) only for the LAST image's own
                    # tiles; an earlier image's trailing tiles consumed in
                    # the same slot keep their batched store triggers.
                    s2_tile(img, fh, t, z1t, fine=fine and img == B - 1)
                pending.pop(0)

        bufa_cur = load_image(0, chunked=True)
        for img in range(B):
            bufa_next = load_image(img + 1) if img + 1 < B else None
            z1t = z1pool.tile([RANK, NPIX], MM_DT, name="z1", tag="z1")
            for t in range(NT2):
                pending.append((img, t, z1t))
            fine = (img == B - 1)
            for q in range(NQ):
                stage1_quad(bufa_cur, q, z1t)
                consume(img, QPX * q, fine=fine)
            bufa_cur = bufa_next
        consume(B - 1, NPIX, fine=True)  # last image's trailing tiles

    nc.compile()
    return nc


def _get_nc():
    if "nc" not in _CACHE:
        _CACHE["nc"] = _build_nc()
    return _CACHE["nc"]


def _prep_inputs(x, k, l_t, s, aux_U, aux_Unp1, aux_Vt, aux_Vtnp1, b, aux_b,
                 step):
    step = int(np.asarray(step))
    x = np.ascontiguousarray(np.asarray(x, dtype=np.float32))
    if step == 0:
        U, W2, bias = np.asarray(k), np.asarray(aux_Vt), np.asarray(aux_b)
    elif step == 1:
        U, W2, bias = np.asarray(aux_U), np.asarray(l_t), np.asarray(aux_b)
    else:
        U = np.asarray(aux_Unp1)
        W2 = (np.asarray(s, np.float64) @ np.asarray(aux_Vtnp1, np.float64))
        bias = np.asarray(b)
    U = U.astype(np.float32)
    W2 = np.ascontiguousarray(W2.astype(MM_NP))
    bias = np.asarray(bias, np.float32)

    # channel-major, zero-padded H and W, fp16
    xpad = np.zeros((B, H + 2, W + 2, C), np.float32)
    xpad[:, 1:-1, 1:-1, :] = x
    xpad_t = np.ascontiguousarray(xpad.transpose(0, 3, 1, 2)).astype(MM_NP)

    # stage-1 stationary slots [128, 6*128]:
    #   p=0..2: top=blocks[p,0], bottom=blocks[p,1] (pairs, K=128)
    #   p=3..5: blocks[p-3,2] duplicated into both halves (concurrent K=64
    #           row-tiles for row-pairs A and B)
    blocks = U.reshape(KH, KW, C, RANK)
    w1p = np.zeros((6, 2 * C, MP), np.float32)
    for p in range(3):
        w1p[p, 0:C, 0:RANK] = blocks[p, 0]
        w1p[p, C:2 * C, 0:RANK] = blocks[p, 1]
    for s_ in range(3):
        w1p[3 + s_, 0:C, 0:RANK] = blocks[s_, 2]
        w1p[3 + s_, C:2 * C, 0:RANK] = blocks[s_, 2]
    w1 = np.ascontiguousarray(
        w1p.transpose(1, 0, 2).reshape(2 * C, 6 * MP)).astype(MM_NP)

    in_maps = []
    for i in range(NCORES):
        xt_i = np.ascontiguousarray(
            xpad_t[:, :, HS * i:HS * i + HSH, :]).reshape(B, C, XL)
        # bias strip -> [f, fh*NPIX + px] (transposed, filter-major)
        bs = bias[HS * i:HS * (i + 1)].reshape(NPIX, FILTERS)
        bt = np.ascontiguousarray(bs.T).astype(MM_NP)      # (256, NPIX)
        b_i = np.ascontiguousarray(
            np.concatenate([bt[0:MP], bt[MP:FILTERS]], axis=1))
        in_maps.append({"xt": xt_i, "w1": w1, "w2": W2, "bias": b_i})
    return in_maps


def _assemble(results):
    strips = [
        results[i]["out"].transpose(0, 3, 1, 2).reshape(B, HS, W, FILTERS)
        for i in range(NCORES)
    ]
    return np.concatenate(strips, axis=1).astype(np.float32)


def run(trace=False, **inputs):
    in_maps = _prep_inputs(**inputs)
    nc = _get_nc()
    res = run_bass_kernel_spmd(nc, in_maps, list(range(NCORES)), trace=trace)
    return _assemble(res.results), res


def kernel(**inputs):
    out, _ = run(trace=False, **inputs)
    return out
